# revision 9
# baseline (speedup 1.0000x reference)
"""MoE layer (top-2 of 8 experts, selection shared across tokens) on 8 TRN2 cores.

Math (faithful to the reference):
    gates = softmax(x @ W_gate + b_gate)          [N, 8]
    idx0  = top-2 expert indices of token 0       [2]
    s     = per-token top-2 gate VALUES (desc)    [N, 2]
    out   = s0 * (x @ W[A] + b[A]) + s1 * (x @ W[B] + b[B])

Strategy: gating + top-2 is 0.2% of the FLOPs -> computed on host.  The two
active expert matmuls (275 GFLOP) are data-parallel sharded over tokens across
8 cores; expert weights are replicated.  Matmuls run in fp16 (values are small,
so fp16 range is safe and its 10-bit mantissa keeps rel-err ~3e-4),
accumulating fp32 in PSUM.

Perf notes (v4):
  - steady-state MM cadence sits at the N=512 streaming floor (215.8ns =
    512/2.4GHz + NX issue); optimization is entirely head/tail/gap removal.
  - DMA is descriptor-run-bound: per-(k,q) tiles (512B runs) move at
    ~140GB/s/queue while blocked 0.5-2MB transfers with >=4KB runs approach
    the HBM-per-core share (~140GB/s/queue, ~280 aggregate during the ramp).
    Host pre-blocks x and W so every load is one contiguous transfer.
  - ramp: the first psum group is supply-bound on (x(q0) 1MB + W(block0)).
    First/last output blocks are 256 wide so that critical set is 3MB, and
    its pieces alternate across both HWDGE queues half-k at a time.
  - the PE clock-gate (HAM) starts at 1.2GHz and needs ~3.4us of sustained
    activity for 2.4GHz; dummy matmuls bridge engine-start -> first real MM.
  - x (8MB fp16) is SBUF-resident, loaded once.  W double-buffers per block;
    the next block's W is prefetched at the END of the previous block so
    output stores never head-of-line-block the queues.  No SWDGE anywhere
    (it added ~5us of teardown drain).
  - last group runs expert A's matmuls before expert B's so half its
    epilogue hides under matmuls; its store splits across both queues.
"""

import functools

import numpy as np

import concourse.bass as bass
import concourse.mybir as mybir
import concourse.tile as tile
from concourse import bacc
from concourse.bass_utils import run_bass_kernel_spmd

N_CORES = 8
N, D_IN, D_HID = 16384, 2048, 2048
NT = N // N_CORES            # tokens per core
KP = 128                     # contraction chunk = partition dim
KCH = D_IN // KP             # 16 K-chunks
NB_WIDTHS = (512, 512, 512, 512)
assert sum(NB_WIDTHS) == D_HID
NBLK = len(NB_WIDTHS)
NB_OFF = tuple(sum(NB_WIDTHS[:i]) for i in range(NBLK))
TQ = 256                     # token slice per resident-x tile
NQ = NT // TQ                # 8 slices
MPQ = TQ // 128              # m-tiles per slice
N_DUMMY = 30                 # HAM warm-up matmuls issued while DMA fills

F32 = mybir.dt.float32
FP16 = mybir.dt.float16

W_DT = FP16
X_DT = FP16

# Filled by test harness inspection: last BassKernelResults from a run.
LAST_RESULT = None


@functools.lru_cache(maxsize=1)
def _build():
    nc = bacc.Bacc("TRN2", target_bir_lowering=False, debug=False)
    # host-blocked layouts: each (q) / (e, nb) slice is contiguous in DRAM
    # xb[q, p, k*TQ+j] = x.T[k*128+p, q*TQ+j]
    xb = nc.dram_tensor("xb", [NQ, 128, KCH * TQ], X_DT, kind="ExternalInput")
    # wf[e, p, KCH*off_b + k*nbw_b + j] = W_e[k*128+p, off_b+j]
    wf = nc.dram_tensor(
        "wf", [2, 128, KCH * D_HID], W_DT, kind="ExternalInput"
    )
    # bias pre-replicated across partitions on host: brep[p, e, o] = b_sel[e, o]
    brep = nc.dram_tensor("brep", [128, 2, D_HID], F32, kind="ExternalInput")
    # per-token scores pre-arranged on host, partition-major:
    # sC[p, m, s] = top2_score[m*128 + p, s]
    sC = nc.dram_tensor("sC", [128, NT // 128, 2], F32, kind="ExternalInput")
    out = nc.dram_tensor("out", [NT, D_HID], F32, kind="ExternalOutput")

    MULT = mybir.AluOpType.mult
    ADD = mybir.AluOpType.add

    with tile.TileContext(nc) as tc:
        with (
            tc.tile_pool(name="cst", bufs=1) as cst,
            tc.tile_pool(name="wp", bufs=2) as wp,
            tc.tile_pool(name="xr", bufs=1) as xr,
            tc.tile_pool(name="ep", bufs=2) as ep,
            tc.tile_pool(name="ps", bufs=4, space=bass.MemorySpace.PSUM) as ps,
        ):
            # HAM warm-up: dummy matmuls into the pa psum ring while the first
            # real operands stream in, so real MMs start at 2.4GHz.
            dm = cst.tile([128, 512], W_DT, tag="dm")
            nc.vector.memset(dm[:], 0.0)
            dps = ps.tile([128, 512], F32, tag="pa", name="dps")
            for _ in range(N_DUMMY):
                nc.tensor.matmul(dps[:], dm[:, 0:128], dm[:], start=True, stop=True)

            x_t = {}
            w_t = {}

            def w_sl(b, k0, k1):
                # dram cols of wf for chunks [k0,k1) of block b
                base = KCH * NB_OFF[b]
                return slice(base + k0 * NB_WIDTHS[b], base + k1 * NB_WIDTHS[b])

            # ── critical-path DMA (block0 + x(q0)), balanced per queue ───
            # sync  : x0.k0-7, Wa0 in k-quarters               (2.5MB)
            # scalar: Wb0.q0, x0.k8-15, Wb0.q1-3, sC, brep0    (2.5MB+)
            nbw0 = NB_WIDTHS[0]
            HKX = KCH // 2 * TQ
            QKW = KCH // 4 * nbw0
            x0 = xr.tile([128, KCH * TQ], X_DT, tag="x0", name="x0")
            wa0 = wp.tile([128, KCH * nbw0], W_DT, tag="w0", name="wa0")
            wb0 = wp.tile([128, KCH * nbw0], W_DT, tag="w1", name="wb0")
            nc.sync.dma_start(x0[:, 0:HKX], xb[0, :, 0:HKX])
            nc.scalar.dma_start(wb0[:, 0:QKW], wf[1, :, w_sl(0, 0, 4)])
            nc.sync.dma_start(wa0[:, 0:QKW], wf[0, :, w_sl(0, 0, 4)])
            nc.scalar.dma_start(x0[:, HKX:], xb[0, :, HKX:])
            for qq in range(1, 4):
                nc.sync.dma_start(
                    wa0[:, qq * QKW:(qq + 1) * QKW],
                    wf[0, :, w_sl(0, 4 * qq, 4 * qq + 4)],
                )
                nc.scalar.dma_start(
                    wb0[:, qq * QKW:(qq + 1) * QKW],
                    wf[1, :, w_sl(0, 4 * qq, 4 * qq + 4)],
                )
            x_t[0] = x0
            w_t[0, 0] = wa0
            w_t[1, 0] = wb0

            sC_sb = cst.tile([128, NT // 128, 2], F32)
            nc.scalar.dma_start(sC_sb[:], sC[:])
            brep_sb = cst.tile([128, 2, D_HID], F32)
            nc.scalar.dma_start(brep_sb[:, :, 0:nbw0], brep[:, :, 0:nbw0])

            for q in range(1, NQ):
                t = xr.tile([128, KCH * TQ], X_DT, tag=f"x{q}", name=f"x{q}")
                (nc.sync if q % 2 == 1 else nc.scalar).dma_start(t[:], xb[q])
                x_t[q] = t

            nc.sync.dma_start(brep_sb[:, :, nbw0:], brep[:, :, nbw0:])

            def prefetch_w(b):
                nbw = NB_WIDTHS[b]
                for e, eng in ((0, nc.sync), (1, nc.scalar)):
                    t = wp.tile(
                        [128, KCH * nbw], W_DT, tag=f"w{e}", name=f"w{e}_{b}",
                        padded_shape=[128, KCH * max(NB_WIDTHS)],
                    )
                    eng.dma_start(t[:], wf[e, :, w_sl(b, 0, KCH)])
                    w_t[e, b] = t

            prefetch_w(1)

            # ── main loop ────────────────────────────────────────────────
            for nb in range(NBLK):
                nbw = NB_WIDTHS[nb]
                nb_sl = slice(NB_OFF[nb], NB_OFF[nb] + nbw)
                wa_c = w_t[0, nb]
                wb_c = w_t[1, nb]
                for q in range(NQ):
                    for mi in range(MPQ):
                        mg = q * MPQ + mi
                        last = nb == NBLK - 1 and mg == NQ * MPQ - 1
                        pa = ps.tile([128, 512], F32, tag="pa", name="pa")[:, :nbw]
                        pb = ps.tile([128, 512], F32, tag="pb", name="pb")[:, :nbw]
                        xq = x_t[q]

                        def xs(k):
                            return xq[:, k * TQ + mi * 128:k * TQ + mi * 128 + 128]

                        def ws(w, k):
                            return w[:, k * nbw:(k + 1) * nbw]

                        if last:
                            # pa finishes its 16 MMs early so half the
                            # epilogue overlaps pb's matmuls
                            for pp, wc in ((pa, wa_c), (pb, wb_c)):
                                for k in range(KCH):
                                    nc.tensor.matmul(
                                        pp[:], xs(k), ws(wc, k),
                                        start=(k == 0), stop=(k == KCH - 1),
                                    )
                        else:
                            for k in range(KCH):
                                nc.tensor.matmul(
                                    pa[:], xs(k), ws(wa_c, k),
                                    start=(k == 0), stop=(k == KCH - 1),
                                )
                                nc.tensor.matmul(
                                    pb[:], xs(k), ws(wb_c, k),
                                    start=(k == 0), stop=(k == KCH - 1),
                                )
                        s0 = sC_sb[:, mg, 0:1]
                        s1 = sC_sb[:, mg, 1:2]
                        # epilogue on DVE: out = s0*(pa+bA) + s1*(pb+bB)
                        # (each op reads at most one PSUM input)
                        u = ep.tile([128, 512], F32, tag="u", name="u")[:, :nbw]
                        nc.vector.tensor_add(u[:], pa[:], brep_sb[:, 0, nb_sl])
                        t1 = ep.tile([128, 512], F32, tag="t1", name="t1")[:, :nbw]
                        nc.vector.tensor_scalar_mul(t1[:], u[:], s0)
                        v = ep.tile([128, 512], F32, tag="v", name="v")[:, :nbw]
                        nc.vector.tensor_add(v[:], pb[:], brep_sb[:, 1, nb_sl])
                        o = ep.tile([128, 512], F32, tag="o", name="o")[:, :nbw]
                        nc.vector.scalar_tensor_tensor(
                            o[:], v[:], s1, t1[:], op0=MULT, op1=ADD
                        )
                        m_sl = bass.ts(mg, 128)
                        if last:
                            # both queues are idle by now; split the store
                            h = nbw // 2
                            nc.sync.dma_start(
                                out[m_sl, nb_sl.start:nb_sl.start + h],
                                o[:, 0:h],
                            )
                            nc.scalar.dma_start(
                                out[m_sl, nb_sl.start + h:nb_sl.stop],
                                o[:, h:],
                            )
                        else:
                            eng = nc.sync if mg % 2 == 0 else nc.scalar
                            eng.dma_start(out[m_sl, nb_sl], o[:])
                # prefetch after this block's stores: the ring slot it waits
                # on frees at this block's last MM, so the queue never stalls
                if nb + 2 <= NBLK - 1:
                    prefetch_w(nb + 2)

    nc.compile()
    return nc


def _host_gating(x, W_gate, b_gate):
    logits = x @ W_gate + b_gate                       # [N, 8] fp32
    m = logits.max(axis=1, keepdims=True)
    e = np.exp(logits - m)
    gates = e / e.sum(axis=1, keepdims=True)
    idx0 = np.argsort(-gates[0], kind="stable")[:2]    # token-0 top-2 experts
    scores = -np.sort(-gates, axis=1)[:, :2]           # per-token top-2 values
    return idx0, np.ascontiguousarray(scores)


def kernel(x, W_experts, b_experts, W_gate, b_gate):
    global LAST_RESULT
    x = np.ascontiguousarray(np.asarray(x, dtype=np.float32))
    W_experts = np.asarray(W_experts, dtype=np.float32)
    b_experts = np.asarray(b_experts, dtype=np.float32)
    W_gate = np.asarray(W_gate, dtype=np.float32)
    b_gate = np.asarray(b_gate, dtype=np.float32)

    idx0, scores = _host_gating(x, W_gate, b_gate)
    w_np_dt = mybir.dt.np(W_DT)
    x_np_dt = mybir.dt.np(X_DT)

    # per-block-contiguous W: wf[e, p, KCH*off_b + k*nbw + j] = W_e[k*128+p, off_b+j]
    w_sel = np.stack([W_experts[idx0[0]], W_experts[idx0[1]]])  # [2, D_IN, D_HID]
    parts = []
    for b, nbw in enumerate(NB_WIDTHS):
        off = NB_OFF[b]
        blk = w_sel[:, :, off:off + nbw]                 # [2, D_IN, nbw]
        parts.append(
            blk.reshape(2, KCH, 128, nbw)
            .transpose(0, 2, 1, 3)
            .reshape(2, 128, KCH * nbw)
        )
    wflat = np.ascontiguousarray(np.concatenate(parts, axis=2)).astype(w_np_dt)

    brep = np.ascontiguousarray(
        np.broadcast_to(b_experts[idx0][None], (128, 2, D_HID))
    ).astype(np.float32)

    xT_full = x.astype(x_np_dt).T                                  # [D_IN, N]

    nc = _build()
    in_maps = []
    for c in range(N_CORES):
        sl = slice(c * NT, (c + 1) * NT)
        # blocked x: [NQ, 128, KCH*TQ]; xbc[q, p, k*TQ+j] = xT[k*128+p, q*TQ+j]
        xbc = np.ascontiguousarray(
            xT_full[:, sl]
            .reshape(KCH, 128, NQ, TQ)
            .transpose(2, 1, 0, 3)
            .reshape(NQ, 128, KCH * TQ)
        )
        in_maps.append(
            {
                "xb": xbc,
                "wf": wflat,
                "brep": brep,
                "sC": np.ascontiguousarray(
                    scores[sl].reshape(NT // 128, 128, 2).transpose(1, 0, 2)
                ),
            }
        )

    res = run_bass_kernel_spmd(nc, in_maps, list(range(N_CORES)))
    LAST_RESULT = res
    return np.concatenate([r["out"] for r in res.results], axis=0)
